# revision 1
# baseline (speedup 1.0000x reference)
"""Trainium2 Bass kernel for one FDM wave-equation step (5-point stencil CNN).

u2 = 2*u1 - u0 + 0.25*lap5(u1) - 0.0025*(j2 - j0)   on (16,1,1024,1024) f32.

Sharding: data-parallel over batch — 2 full images per NeuronCore, so no halo
exchange is needed. Per core, each image is processed in 9 row-tiles of <=126
output rows. The vertical part of the stencil (which crosses SBUF partitions)
is computed on the TensorEngine as a banded-matrix matmul over the tile's u1
row window; u0 is folded into the same PSUM accumulation via a -I matmul, and
the tile's missing top-neighbor row rides along in that matmul (stashed at
partition M of the u0 tile, with a C_LAP entry at [M, 0] of the matrix). The
horizontal stencil and the j2/j0 terms are fused scalar_tensor_tensor ops on
the VectorEngine (the shift ops run in-place, which also gives correct
zero-padding at the left/right image edges for free).
"""

import numpy as np

import concourse.bacc as bacc
import concourse.mybir as mybir
import concourse.tile as tile
from concourse import bass_utils

F32 = mybir.dt.float32
ALU = mybir.AluOpType

H = W = 1024
B = 16
NCORES = 8
IMGS_PER_CORE = B // NCORES          # 2
ROWS = IMGS_PER_CORE * H             # 2048 rows per core
TS = 126                             # output rows per tile
NTILES = (H + TS - 1) // TS          # 9
M_LAST = H - TS * (NTILES - 1)       # 16

C_LAP = 0.25                         # (DT*C/DX)^2
C_J = 0.0025                         # DT / (2*EPSILON)
C_CENTER = 2.0 - 4.0 * C_LAP         # 1.0


def _const_matrices():
    # bandA[k, m]: weight of u1-window partition k (image row base+k) on
    # output row m.
    bandA = np.zeros((128, 128), dtype=np.float32)
    for m in range(128):
        if m >= 1:
            bandA[m - 1, m] = C_LAP
        bandA[m, m] = C_CENTER
        if m + 1 < 128:
            bandA[m + 1, m] = C_LAP
    negi = -np.eye(128, dtype=np.float32)
    # Variants with the top-neighbor row (stashed at partition M) feeding
    # output row 0.
    negix126 = negi.copy()
    negix126[126, 0] = C_LAP
    negix16 = negi.copy()
    negix16[16, 0] = C_LAP
    return bandA, negi, negix126, negix16


def _build_program():
    nc = bacc.Bacc(
        "TRN2",
        debug=False,
        enable_asserts=False,
        target_bir_lowering=False,
        num_devices=NCORES,
    )
    u1d = nc.dram_tensor("u1", [ROWS, W], F32, kind="ExternalInput").ap()
    u0d = nc.dram_tensor("u0", [ROWS, W], F32, kind="ExternalInput").ap()
    j2d = nc.dram_tensor("j2", [ROWS, W], F32, kind="ExternalInput").ap()
    j0d = nc.dram_tensor("j0", [ROWS, W], F32, kind="ExternalInput").ap()
    outd = nc.dram_tensor("out", [ROWS, W], F32, kind="ExternalOutput").ap()

    consts_np = _const_matrices()
    names = ["bandA", "negi", "negix126", "negix16"]
    const_d = [nc.inline_tensor(m, name=n) for m, n in zip(consts_np, names)]

    with tile.TileContext(nc) as tc:
        with tc.tile_pool(name="consts", bufs=1) as cpool, \
             tc.tile_pool(name="io", bufs=9) as iopool, \
             tc.tile_pool(name="res", bufs=6) as rpool, \
             tc.tile_pool(name="ps", bufs=3, space="PSUM") as pspool:
            csb = [cpool.tile([128, 128], F32, name=f"{n}_sb")
                   for n in names]
            band_sb, negi_sb, negix126_sb, negix16_sb = csb
            consts_loaded = False

            for img in range(IMGS_PER_CORE):
                r0 = H * img
                for t in range(NTILES):
                    base = TS * t
                    M = min(TS, H - base)
                    K1 = min(M + 1, H - base)    # u1 window rows (incl. bottom nbr)

                    u1t = iopool.tile([128, W], F32, name="u1t")
                    nc.sync.dma_start(u1t[0:K1], u1d[r0 + base:r0 + base + K1, :])
                    u0t = iopool.tile([128, W], F32, name="u0t")
                    nc.sync.dma_start(u0t[0:M], u0d[r0 + base:r0 + base + M, :])
                    if t == 0:
                        K2, nmat = M, negi_sb
                    else:
                        # top-neighbor u1 row rides at partition M
                        # (tiny 4 KiB DMA: keep it off the busy HWDGE rings)
                        nc.gpsimd.dma_start(
                            u0t[M:M + 1], u1d[r0 + base - 1:r0 + base, :]
                        )
                        K2 = M + 1
                        nmat = negix126_sb if M == 126 else negix16_sb
                    if not consts_loaded:
                        # const loads issued after the first big loads so the
                        # sync ring's first descriptor-gen feeds data at once
                        for d, sb in zip(const_d, csb):
                            nc.sync.dma_start(sb[:], d.ap())
                        consts_loaded = True
                    j2t = iopool.tile([128, W], F32, name="j2t")
                    nc.scalar.dma_start(j2t[0:M], j2d[r0 + base:r0 + base + M, :])
                    j0t = iopool.tile([128, W], F32, name="j0t")
                    nc.scalar.dma_start(j0t[0:M], j0d[r0 + base:r0 + base + M, :])

                    # PSUM accumulates: band@u1 - u0 (+top-neighbor row).
                    ps = pspool.tile([128, W], F32, name="ps")
                    for h in range(2):
                        cs = slice(512 * h, 512 * h + 512)
                        nc.tensor.matmul(
                            ps[0:M, cs], band_sb[0:K1, 0:M], u1t[0:K1, cs],
                            start=True, stop=False,
                        )
                        nc.tensor.matmul(
                            ps[0:M, cs], nmat[0:K2, 0:M], u0t[0:K2, cs],
                            start=False, stop=True,
                        )

                    rt = rpool.tile([128, W], F32, name="rt")
                    # rt = -C_J*j2 + ps   (split per PSUM bank: the first half
                    # can start while the second bank's matmuls still run)
                    for h in range(2):
                        cs = slice(512 * h, 512 * h + 512)
                        nc.vector.scalar_tensor_tensor(
                            rt[0:M, cs], j2t[0:M, cs], -C_J, ps[0:M, cs],
                            ALU.mult, ALU.add,
                        )
                    # rt += C_J*j0
                    nc.vector.scalar_tensor_tensor(
                        rt[0:M, :], j0t[0:M, :], C_J, rt[0:M, :],
                        ALU.mult, ALU.add,
                    )
                    # rt[:, 1:] += C_LAP * u1[., x-1]  (left neighbor)
                    nc.vector.scalar_tensor_tensor(
                        rt[0:M, 1:W], u1t[0:M, 0:W - 1], C_LAP,
                        rt[0:M, 1:W], ALU.mult, ALU.add,
                    )
                    # rt[:, :1023] += C_LAP * u1[., x+1]  (right neighbor)
                    nc.vector.scalar_tensor_tensor(
                        rt[0:M, 0:W - 1], u1t[0:M, 1:W], C_LAP,
                        rt[0:M, 0:W - 1], ALU.mult, ALU.add,
                    )

                    nc.scalar.dma_start(outd[r0 + base:r0 + base + M, :], rt[0:M, :])

    nc.compile()
    return nc


_NC_CACHE = None


def _get_program():
    global _NC_CACHE
    if _NC_CACHE is None:
        _NC_CACHE = _build_program()
    return _NC_CACHE


def kernel(u1, u0, j2, j0):
    nc = _get_program()
    in_maps = []
    for c in range(NCORES):
        sl = slice(IMGS_PER_CORE * c, IMGS_PER_CORE * (c + 1))
        in_maps.append({
            "u1": np.ascontiguousarray(u1[sl]).reshape(ROWS, W),
            "u0": np.ascontiguousarray(u0[sl]).reshape(ROWS, W),
            "j2": np.ascontiguousarray(j2[sl]).reshape(ROWS, W),
            "j0": np.ascontiguousarray(j0[sl]).reshape(ROWS, W),
        })
    res = bass_utils.run_bass_kernel_spmd(nc, in_maps, core_ids=list(range(NCORES)))
    out = np.concatenate(
        [r["out"].reshape(IMGS_PER_CORE, 1, H, W) for r in res.results], axis=0
    )
    return out.astype(np.float32, copy=False)



# revision 6
# speedup vs baseline: 1.3464x; 1.3464x over previous
"""Trainium2 Bass kernel for one FDM wave-equation step (5-point stencil CNN).

u2 = 2*u1 - u0 + 0.25*lap5(u1) - 0.0025*(j2 - j0)   on (16,1,1024,1024) f32.

Sharding: data-parallel over batch - 2 full images per NeuronCore, no halo
exchange between cores.

The kernel is DMA-bandwidth-bound (all engines share one DMA path), so HBM
traffic is minimized by casting inputs during the load DMA (SWDGE on the
gpsimd queue can cast): u1/u0 load as fp16, j2/j0 as fp8e4 (the j terms are
scaled by 0.0025, so their contribution to the output is ~0.2% and fp8
quantization error is negligible). The f32 output store is the only
full-precision transfer. Loads are batched one-DMA-per-image-per-tensor
(9 gpsimd DMA instructions per core per image including the 2 tiny halo-row
loads) to amortize SWDGE descriptor-generation overhead.

All arithmetic runs on the TensorEngine as PSUM-accumulated matmuls over
128-row blocks: a tridiagonal band matrix gives the vertical stencil +
center term, -I folds in u0, +/-0.0025*I fold in j0/j2 (fp16 stationary x
fp8 moving), the horizontal stencil is two 0.25*I matmuls with the moving
AP shifted by one column (which also yields correct zero padding at image
edges), and the rows adjacent to each 128-row block boundary get their
cross-block vertical neighbor from a small halo tile via a 9-partition
selector matmul. The Activation engine copies PSUM->SBUF (f32) and the SP
queue stores the result rows to HBM.
"""

import numpy as np

import concourse.bacc as bacc
import concourse.mybir as mybir
import concourse.tile as tile
from concourse import bass_utils

F32 = mybir.dt.float32
F16 = mybir.dt.float16
FP8 = mybir.dt.float8e4
ACT = mybir.ActivationFunctionType

H = W = 1024
B = 16
NCORES = 8
IMGS_PER_CORE = B // NCORES          # 2
ROWS = IMGS_PER_CORE * H             # 2048 rows per core
TB = 128                             # block rows (= partition dim)
NT = H // TB                         # 8 blocks per image

C_LAP = 0.25                         # (DT*C/DX)^2
C_J = 0.0025                         # DT / (2*EPSILON)
C_CENTER = 2.0 - 4.0 * C_LAP         # 1.0


def _const_matrices():
    # lhsT layout [K, M]: weight of moving-tensor partition k on out row m.
    band = np.zeros((128, 128), dtype=np.float16)
    for m in range(128):
        band[m, m] = C_CENTER
        if m >= 1:
            band[m - 1, m] = C_LAP
        if m + 1 < 128:
            band[m + 1, m] = C_LAP
    negi = (-np.eye(128)).astype(np.float16)
    qi = (C_LAP * np.eye(128)).astype(np.float16)
    jp = (C_J * np.eye(128)).astype(np.float16)
    jm = (-C_J * np.eye(128)).astype(np.float16)
    # halo selectors: rhs is always halosb[0:16] (moving-tensor base
    # partition must be 0); halo partition t holds block t's top neighbor
    # row (feeds out row 0), partition 8+t its bottom neighbor (out row 127).
    out = {"band": band, "negi": negi, "qi": qi, "jp": jp, "jm": jm}
    for t in range(8):
        hsel = np.zeros((16, 128), dtype=np.float16)
        if t >= 1:
            hsel[t, 0] = C_LAP
        if t <= 6:
            hsel[8 + t, 127] = C_LAP
        out[f"hsel{t}"] = hsel
    return out


def _build_program():
    nc = bacc.Bacc(
        "TRN2",
        debug=False,
        enable_asserts=False,
        target_bir_lowering=False,
        num_devices=NCORES,
    )
    u1d = nc.dram_tensor("u1", [ROWS, W], F32, kind="ExternalInput").ap()
    u0d = nc.dram_tensor("u0", [ROWS, W], F32, kind="ExternalInput").ap()
    j2d = nc.dram_tensor("j2", [ROWS, W], F32, kind="ExternalInput").ap()
    j0d = nc.dram_tensor("j0", [ROWS, W], F32, kind="ExternalInput").ap()
    outd = nc.dram_tensor("out", [ROWS, W], F32, kind="ExternalOutput").ap()

    consts_np = _const_matrices()
    const_d = {n: nc.inline_tensor(m, name=n) for n, m in consts_np.items()}

    with tile.TileContext(nc) as tc:
        with tc.tile_pool(name="consts", bufs=1) as cpool, \
             tc.tile_pool(name="io", bufs=2) as iopool, \
             tc.tile_pool(name="res", bufs=3) as rpool, \
             tc.tile_pool(name="ps", bufs=3, space="PSUM") as pspool:
            csb = {}
            for n, m in consts_np.items():
                csb[n] = cpool.tile(list(m.shape), F16, name=f"{n}_sb")

            consts_loaded = False
            for img in range(IMGS_PER_CORE):
                r0 = H * img
                img_slice = slice(r0, r0 + H)
                u1r = u1d[img_slice, :].rearrange("(t p) c -> p t c", p=TB)
                u0r = u0d[img_slice, :].rearrange("(t p) c -> p t c", p=TB)
                j2r = j2d[img_slice, :].rearrange("(t p) c -> p t c", p=TB)
                j0r = j0d[img_slice, :].rearrange("(t p) c -> p t c", p=TB)
                outr = outd[img_slice, :].rearrange("(t p) c -> p t c", p=TB)

                u1sb = iopool.tile([128, NT, W], F16, name="u1sb")
                nc.gpsimd.dma_start(u1sb[:], u1r)
                u0sb = iopool.tile([128, NT, W], F16, name="u0sb")
                nc.gpsimd.dma_start(u0sb[:], u0r)
                # halo rows: partition 1+t = u1 row 128*(t+1)-1 (top halo of
                # block t+1), partition 8+t = u1 row 128*(t+1) (bottom halo of
                # block t), t = 0..6.
                halosb = iopool.tile([16, W], F16, name="halosb")
                # partitions 0 and 15 are never loaded but are read (with
                # zero weight) by the halo matmuls - keep them finite
                nc.vector.memset(halosb[:], 0.0)
                nc.gpsimd.dma_start(
                    halosb[1:NT, :], u1d[r0 + TB - 1:r0 + H - TB:TB, :])
                nc.gpsimd.dma_start(
                    halosb[NT:2 * NT - 1, :], u1d[r0 + TB:r0 + H - TB + 1:TB, :])
                if not consts_loaded:
                    # issued after the first big loads so the SWDGE ring has
                    # data to chew on immediately
                    for n in consts_np:
                        nc.sync.dma_start(csb[n][:], const_d[n].ap())
                    consts_loaded = True
                j2sb = iopool.tile([128, NT, W], FP8, name="j2sb")
                nc.gpsimd.dma_start(j2sb[:], j2r)
                j0sb = iopool.tile([128, NT, W], FP8, name="j0sb")
                nc.gpsimd.dma_start(j0sb[:], j0r)

                for t in range(NT):
                    ps = pspool.tile([128, W], F32, name="ps")
                    rt = rpool.tile([128, W], F32, name="rt")
                    for h in range(2):
                        c0 = 512 * h
                        cs = slice(c0, c0 + 512)
                        mm = nc.tensor.matmul
                        mm(ps[:, cs], csb["band"][:], u1sb[:, t, cs],
                           start=True, stop=False)
                        mm(ps[:, cs], csb["negi"][:], u0sb[:, t, cs],
                           start=False, stop=False)
                        # horizontal stencil: moving AP shifted one column
                        lo = max(c0 - 1, 0)
                        mm(ps[:, lo + 1:c0 + 512], csb["qi"][:],
                           u1sb[:, t, lo:c0 + 511], start=False, stop=False)
                        hi = min(c0 + 513, W)
                        mm(ps[:, c0:hi - 1], csb["qi"][:],
                           u1sb[:, t, c0 + 1:hi], start=False, stop=False)
                        mm(ps[:, cs], csb["jp"][:], j0sb[:, t, cs],
                           start=False, stop=False)
                        mm(ps[:, cs], csb["jm"][:], j2sb[:, t, cs],
                           start=False, stop=False)
                        mm(ps[:, cs], csb[f"hsel{t}"][:], halosb[0:16, cs],
                           start=False, stop=True)
                        nc.scalar.activation(rt[:, cs], ps[:, cs], ACT.Copy)
                    nc.sync.dma_start(outr[:, t, :], rt[:])

    nc.compile()
    return nc


_NC_CACHE = None


def _get_program():
    global _NC_CACHE
    if _NC_CACHE is None:
        _NC_CACHE = _build_program()
    return _NC_CACHE


def kernel(u1, u0, j2, j0):
    nc = _get_program()
    in_maps = []
    for c in range(NCORES):
        sl = slice(IMGS_PER_CORE * c, IMGS_PER_CORE * (c + 1))
        in_maps.append({
            "u1": np.ascontiguousarray(u1[sl]).reshape(ROWS, W),
            "u0": np.ascontiguousarray(u0[sl]).reshape(ROWS, W),
            "j2": np.ascontiguousarray(j2[sl]).reshape(ROWS, W),
            "j0": np.ascontiguousarray(j0[sl]).reshape(ROWS, W),
        })
    res = bass_utils.run_bass_kernel_spmd(nc, in_maps, core_ids=list(range(NCORES)))
    out = np.concatenate(
        [r["out"].reshape(IMGS_PER_CORE, 1, H, W) for r in res.results], axis=0
    )
    return out.astype(np.float32, copy=False)


# revision 9
# speedup vs baseline: 1.5711x; 1.1669x over previous
"""Trainium2 Bass kernel for one FDM wave-equation step (5-point stencil CNN).

u2 = 2*u1 - u0 + 0.25*lap5(u1) - 0.0025*(j2 - j0)   on (16,1,1024,1024) f32.

Sharding: data-parallel over batch - 2 full images per NeuronCore, no halo
exchange between cores.

The kernel is DMA-bandwidth-bound (all engines share one DMA path), so HBM
traffic is minimized by casting inputs during the load DMA (SWDGE on the
gpsimd queue can cast): u1/u0 load as fp16, j2/j0 as fp8e4 (the j terms are
scaled by 0.0025, so their contribution to the output is ~0.2% and fp8
quantization error is negligible). The f32 output store is the only
full-precision transfer. Loads are batched one-DMA-per-image-per-tensor
(9 gpsimd DMA instructions per core per image including the 2 tiny halo-row
loads) to amortize SWDGE descriptor-generation overhead.

All arithmetic runs on the TensorEngine as PSUM-accumulated matmuls over
128-row blocks: a tridiagonal band matrix gives the vertical stencil +
center term, -I folds in u0, +/-0.0025*I fold in j0/j2 (fp16 stationary x
fp8 moving), the horizontal stencil is two 0.25*I matmuls with the moving
AP shifted by one column (which also yields correct zero padding at image
edges), and the rows adjacent to each 128-row block boundary get their
cross-block vertical neighbor from a small halo tile via a 9-partition
selector matmul. The Activation engine copies PSUM->SBUF (f32) and the SP
queue stores the result rows to HBM.
"""

import numpy as np

import concourse.bacc as bacc
import concourse.mybir as mybir
import concourse.tile as tile
from concourse import bass_utils

F32 = mybir.dt.float32
F16 = mybir.dt.float16
FP8 = mybir.dt.float8e4
ACT = mybir.ActivationFunctionType

H = W = 1024
B = 16
NCORES = 8
IMGS_PER_CORE = B // NCORES          # 2
ROWS = IMGS_PER_CORE * H             # 2048 rows per core
TB = 128                             # block rows (= partition dim)
NT = H // TB                         # 8 blocks per image

C_LAP = 0.25                         # (DT*C/DX)^2
C_J = 0.0025                         # DT / (2*EPSILON)
C_CENTER = 2.0 - 4.0 * C_LAP         # 1.0


def _const_matrices():
    # lhsT layout [K, M]: weight of moving-tensor partition k on out row m.
    band = np.zeros((128, 128), dtype=np.float16)
    for m in range(128):
        band[m, m] = C_CENTER
        if m >= 1:
            band[m - 1, m] = C_LAP
        if m + 1 < 128:
            band[m + 1, m] = C_LAP
    negi = (-np.eye(128)).astype(np.float16)
    jp = (C_J * np.eye(128)).astype(np.float16)
    jm = (-C_J * np.eye(128)).astype(np.float16)
    # halo selectors: rhs is always halosb[0:16] (moving-tensor base
    # partition must be 0); halo partition t holds block t's top neighbor
    # row (feeds out row 0), partition 8+t its bottom neighbor (out row 127).
    out = {"band": band, "negi": negi, "jp": jp, "jm": jm}
    for t in range(8):
        hsel = np.zeros((16, 128), dtype=np.float16)
        if t >= 1:
            hsel[t, 0] = C_LAP
        if t <= 6:
            hsel[8 + t, 127] = C_LAP
        out[f"hsel{t}"] = hsel
    return out


def _build_program():
    nc = bacc.Bacc(
        "TRN2",
        debug=False,
        enable_asserts=False,
        target_bir_lowering=False,
        num_devices=NCORES,
    )
    u1d = nc.dram_tensor("u1", [ROWS, W], F32, kind="ExternalInput").ap()
    u0d = nc.dram_tensor("u0", [ROWS, W], F32, kind="ExternalInput").ap()
    j2d = nc.dram_tensor("j2", [ROWS, W], F32, kind="ExternalInput").ap()
    j0d = nc.dram_tensor("j0", [ROWS, W], F32, kind="ExternalInput").ap()
    outd = nc.dram_tensor("out", [ROWS, W], F32, kind="ExternalOutput").ap()

    consts_np = _const_matrices()
    const_d = {n: nc.inline_tensor(m, name=n) for n, m in consts_np.items()}

    with tile.TileContext(nc) as tc:
        with tc.tile_pool(name="consts", bufs=1) as cpool, \
             tc.tile_pool(name="io", bufs=2) as iopool, \
             tc.tile_pool(name="res", bufs=3) as rpool, \
             tc.tile_pool(name="ps", bufs=3, space="PSUM") as pspool:
            csb = {}
            for n, m in consts_np.items():
                csb[n] = cpool.tile(list(m.shape), F16, name=f"{n}_sb")

            consts_loaded = False
            for img in range(IMGS_PER_CORE):
                r0 = H * img
                img_slice = slice(r0, r0 + H)
                u1r = u1d[img_slice, :].rearrange("(t p) c -> p t c", p=TB)
                u0r = u0d[img_slice, :].rearrange("(t p) c -> p t c", p=TB)
                j2r = j2d[img_slice, :].rearrange("(t p) c -> p t c", p=TB)
                j0r = j0d[img_slice, :].rearrange("(t p) c -> p t c", p=TB)
                outr = outd[img_slice, :].rearrange("(t p) c -> p t c", p=TB)

                u1sb = iopool.tile([128, NT, W], F16, name="u1sb")
                nc.gpsimd.dma_start(u1sb[:], u1r)
                u0sb = iopool.tile([128, NT, W], F16, name="u0sb")
                nc.gpsimd.dma_start(u0sb[:], u0r)
                # halo rows: partition 1+t = u1 row 128*(t+1)-1 (top halo of
                # block t+1), partition 8+t = u1 row 128*(t+1) (bottom halo of
                # block t), t = 0..6.
                halosb = iopool.tile([16, W], F16, name="halosb")
                # partitions 0 and 15 are never loaded but are read (with
                # zero weight) by the halo matmuls - keep them finite
                nc.vector.memset(halosb[:], 0.0)
                nc.gpsimd.dma_start(
                    halosb[1:NT, :], u1d[r0 + TB - 1:r0 + H - TB:TB, :])
                nc.gpsimd.dma_start(
                    halosb[NT:2 * NT - 1, :], u1d[r0 + TB:r0 + H - TB + 1:TB, :])
                if not consts_loaded:
                    # issued after the first big loads so the SWDGE ring has
                    # data to chew on immediately
                    for n in consts_np:
                        nc.sync.dma_start(csb[n][:], const_d[n].ap())
                    consts_loaded = True
                j2sb = iopool.tile([128, NT, W], FP8, name="j2sb")
                nc.gpsimd.dma_start(j2sb[:], j2r)
                j0sb = iopool.tile([128, NT, W], FP8, name="j0sb")
                nc.gpsimd.dma_start(j0sb[:], j0r)

                for t in range(NT):
                    ps = pspool.tile([128, W], F32, name="ps")
                    rt = rpool.tile([128, W], F32, name="rt")
                    for h in range(2):
                        c0 = 512 * h
                        cs = slice(c0, c0 + 512)
                        mm = nc.tensor.matmul
                        mm(ps[:, cs], csb["band"][:], u1sb[:, t, cs],
                           start=True, stop=False)
                        mm(ps[:, cs], csb["negi"][:], u0sb[:, t, cs],
                           start=False, stop=False)
                        mm(ps[:, cs], csb["jp"][:], j0sb[:, t, cs],
                           start=False, stop=False)
                        mm(ps[:, cs], csb["jm"][:], j2sb[:, t, cs],
                           start=False, stop=False)
                        mm(ps[:, cs], csb[f"hsel{t}"][:], halosb[0:16, cs],
                           start=False, stop=True)
                    # horizontal stencil + PSUM->SBUF on DVE/Act:
                    # right neighbor fused with the PSUM read ...
                    stt = nc.vector.scalar_tensor_tensor
                    ALU = mybir.AluOpType
                    stt(rt[:, 0:512], u1sb[:, t, 1:513], C_LAP,
                        ps[:, 0:512], ALU.mult, ALU.add)
                    stt(rt[:, 512:W - 1], u1sb[:, t, 513:W], C_LAP,
                        ps[:, 512:W - 1], ALU.mult, ALU.add)
                    # ... image's last column has no right neighbor
                    nc.scalar.activation(rt[:, W - 1:W], ps[:, W - 1:W],
                                         ACT.Copy)
                    # ... then accumulate the left neighbor
                    stt(rt[:, 1:512], u1sb[:, t, 0:511], C_LAP,
                        rt[:, 1:512], ALU.mult, ALU.add)
                    stt(rt[:, 512:W], u1sb[:, t, 511:W - 1], C_LAP,
                        rt[:, 512:W], ALU.mult, ALU.add)
                    nc.sync.dma_start(outr[:, t, :], rt[:])

    nc.compile()
    return nc


_NC_CACHE = None


def _get_program():
    global _NC_CACHE
    if _NC_CACHE is None:
        _NC_CACHE = _build_program()
    return _NC_CACHE


def kernel(u1, u0, j2, j0):
    nc = _get_program()
    in_maps = []
    for c in range(NCORES):
        sl = slice(IMGS_PER_CORE * c, IMGS_PER_CORE * (c + 1))
        in_maps.append({
            "u1": np.ascontiguousarray(u1[sl]).reshape(ROWS, W),
            "u0": np.ascontiguousarray(u0[sl]).reshape(ROWS, W),
            "j2": np.ascontiguousarray(j2[sl]).reshape(ROWS, W),
            "j0": np.ascontiguousarray(j0[sl]).reshape(ROWS, W),
        })
    res = bass_utils.run_bass_kernel_spmd(nc, in_maps, core_ids=list(range(NCORES)))
    out = np.concatenate(
        [r["out"].reshape(IMGS_PER_CORE, 1, H, W) for r in res.results], axis=0
    )
    return out.astype(np.float32, copy=False)
